# revision 1
# baseline (speedup 1.0000x reference)
"""BitLinear (ternary-weight / int8-activation quantized linear) on 8 trn2 NeuronCores.

Math (matches the jax reference up to fp32 rounding):
    eta   = clip(max|x| along k, 1e-5)             per row
    x_q   = round(x * 127 / eta)    in [-127,127]  (round-half-even)
    gamma = clip(mean|w|, 1e-5)                    scalar
    w_q   = round(clip(w / gamma, -1, 1))          in {-1,0,1}
    out   = (x_q @ w_q^T) * (eta/127 * gamma) + bias

x_q / w_q are small integers exactly representable in bf16 and the PE
accumulates in fp32, so the bf16 matmul is EXACT.  Rounding uses the fp32
magic-number trick  rint(t) = (t + 1.5*2^23) - 1.5*2^23  (round-half-even).

Sharding: data-parallel over rows of x (16384 -> 2048 rows/core), weight+bias
replicated.  Per-core schedule:
  phase W: stream w once (both HWDGE queues), fused |w| reduce -> gamma,
           quantize, PE-transpose into k-major wqT (SBUF resident, bf16)
  phase X: stream x, per-row eta, quantize, round-trip x_q through DRAM;
           m-block 0 transposed on PE, blocks 1-3 via DMA-xbar transposed
           loads that overlap the matmul phase
  phase MM: 1024 bf16 matmuls (k-contiguous per m-tile), ACT dequant-scale
           from PSUM, DVE bias add, stores on the sync queue
"""

import os
from contextlib import ExitStack

import numpy as np
import ml_dtypes

import concourse.bass as bass
import concourse.bacc as bacc
import concourse.mybir as mybir
import concourse.tile as tile
from concourse.bass_utils import run_bass_kernel_spmd

P = 128
K = 2048
N = 2048
M_CORE = 2048
KT = K // P          # 16
NT = N // P          # 16
MT = M_CORE // P     # 16
NBLK = N // 512      # 4
N_CORES = 8
C_MAGIC = 12582912.0     # 1.5 * 2**23
INV_NK = 1.0 / (N * K)

F32 = mybir.dt.float32
BF16 = mybir.dt.bfloat16
ALU = mybir.AluOpType
AXIS = mybir.AxisListType
ACTF = mybir.ActivationFunctionType


def _build_program():
    nc = bacc.Bacc("TRN2", target_bir_lowering=False, debug=False)

    x_d = nc.dram_tensor("x", [M_CORE, K], F32, kind="ExternalInput").ap()
    w_d = nc.dram_tensor("weight", [N, K], F32, kind="ExternalInput").ap()
    b_d = nc.dram_tensor("bias", [1, N], F32, kind="ExternalInput").ap()
    out_d = nc.dram_tensor("out", [M_CORE, N], F32, kind="ExternalOutput").ap()
    xq_rt_d = nc.dram_tensor("xq_rt", [M_CORE, K], BF16).ap()
    ident_d = nc.inline_tensor(
        np.eye(P, dtype=ml_dtypes.bfloat16), name="ident128"
    ).ap()
    identf_d = nc.inline_tensor(
        np.eye(P, dtype=np.float32), name="ident128f"
    ).ap()

    with tile.TileContext(nc) as tc, ExitStack() as ctx:
        consts = ctx.enter_context(tc.tile_pool(name="consts", bufs=1))
        stats = ctx.enter_context(tc.tile_pool(name="stats", bufs=1))
        wqT_p = ctx.enter_context(tc.tile_pool(name="wqT", bufs=1))
        ps_tr = ctx.enter_context(
            tc.tile_pool(name="pstr", bufs=2, space=bass.MemorySpace.PSUM)
        )
        ps_mm = ctx.enter_context(
            tc.tile_pool(name="psmm", bufs=5, space=bass.MemorySpace.PSUM)
        )

        # ---- constants / stats ----
        ident_sb = consts.tile([P, P], BF16)
        nc.sync.dma_start(ident_sb[:], ident_d[:, :])
        ones128 = consts.tile([P, P], F32)
        nc.vector.memset(ones128[:], 1.0)
        ident_f32 = consts.tile([P, P], F32)
        nc.sync.dma_start(ident_f32[:], identf_d[:, :])

        eta_raw = stats.tile([P, MT], F32)
        eta_all = stats.tile([P, MT], F32)
        inv_eta = stats.tile([P, MT], F32)
        qs_all = stats.tile([P, MT], F32)
        osc_all = stats.tile([P, MT], F32)
        wparts = stats.tile([P, NT], F32)
        wsum = stats.tile([P, 1], F32)
        gamma = stats.tile([P, 1], F32)
        inv_g = stats.tile([P, 1], F32)

        # k-major quantized operands, single big tiles:
        # layout [128 k-part, kt*2048 + col]
        wqT_all = wqT_p.tile([P, KT * N], BF16)
        wqT_3d = wqT_all[:].rearrange("p (t n) -> p t n", t=KT)

        # ============ gamma: streamed |w| reduce (pass 1, tiles discarded) ====
        with tc.tile_pool(name="w1stage", bufs=4) as w1stage:
            for nt in range(NT):
                t = w1stage.tile([P, K], F32, tag="w1", name=f"w1_{nt}")
                eng = nc.sync if nt % 2 == 0 else nc.scalar
                eng.dma_start(t[:], w_d[nt * P:(nt + 1) * P, :])
                nc.vector.tensor_reduce(
                    wparts[:, nt:nt + 1], t[:], axis=AXIS.X, op=ALU.add,
                    apply_absolute_value=True,
                )
        nc.vector.tensor_reduce(wsum[:], wparts[:], axis=AXIS.X, op=ALU.add)
        pg = ps_mm.tile([P, 1], F32, tag="psg", name="psg", bufs=1)
        nc.tensor.matmul(pg[:], ones128[:, :], wsum[:])
        nc.vector.tensor_scalar(
            gamma[:], pg[:], scalar1=INV_NK, scalar2=1e-5,
            op0=ALU.mult, op1=ALU.max,
        )
        nc.vector.reciprocal(inv_g[:], gamma[:])

        # =================== phase W (streamed quantize) ===================
        with tc.tile_pool(name="wstage", bufs=5) as wstage, \
             tc.tile_pool(name="wqst", bufs=3) as wqst:
            for nt in range(NT):
                t = wstage.tile([P, K], F32, tag="w", name=f"w{nt}")
                eng = nc.sync if nt % 2 == 0 else nc.scalar
                eng.dma_start(t[:], w_d[nt * P:(nt + 1) * P, :])
                # t = w/gamma + C on ACT (fp32 store rounds to the integer
                # grid; round-then-clip == clip-then-round for this quantizer)
                nc.scalar.activation(
                    t[:], t[:], ACTF.Copy, bias=C_MAGIC, scale=inv_g[:, :]
                )
                nc.vector.tensor_scalar(
                    t[:], t[:], scalar1=C_MAGIC, scalar2=1.0,
                    op0=ALU.subtract, op1=ALU.min,
                )
                q = wqst.tile([P, K], BF16, tag="wq", name=f"wq{nt}")
                nc.vector.tensor_scalar(
                    q[:], t[:], scalar1=-1.0, scalar2=None, op0=ALU.max,
                )
                for g in range(4):
                    pt = ps_tr.tile([P, 512], BF16, tag="ptr", name=f"wt{nt}_{g}")
                    for j in range(4):
                        kt = g * 4 + j
                        nc.tensor.transpose(
                            pt[:, j * P:(j + 1) * P],
                            q[:, kt * P:(kt + 1) * P],
                            ident_sb[:],
                        )
                    dst = wqT_3d[:, g * 4:(g + 1) * 4, nt * P:(nt + 1) * P]
                    src = pt[:].rearrange("p (j n) -> p j n", j=4)
                    if g % 2 == 0:
                        nc.scalar.copy(dst, src)
                    else:
                        nc.vector.tensor_copy(dst, src)

        # =================== phase X + MM (pipelined) ===================
        with tc.tile_pool(name="xqT", bufs=1) as xqT_p, \
             tc.tile_pool(name="xstage", bufs=4) as xstage, \
             tc.tile_pool(name="xqst", bufs=4) as xqst, \
             tc.tile_pool(name="bias_p", bufs=1) as bias_p, \
             tc.tile_pool(name="outst", bufs=3) as outst:
            xqT_all = xqT_p.tile([P, KT * M_CORE], BF16)
            xqT_3d = xqT_all[:].rearrange("p (t m) -> p t m", t=KT)
            b_row = bias_p.tile([1, N], F32)
            nc.sync.dma_start(b_row[:], b_d[:, :])
            b_bf = bias_p.tile([1, N], BF16)
            nc.vector.tensor_copy(b_bf[:], b_row[:])
            # per-mt inv_osc rows (bf16, on partition 0) for the bias matmul
            ios_row = bias_p.tile([1, MT * P], BF16)
            ios_f32 = bias_p.tile([1, MT * P], F32)

            xq_tiles = {}

            def x_iter(mt, store_rt):
                t = xstage.tile([P, K], F32, tag="x", name=f"x{mt}")
                nc.gpsimd.dma_start(t[:], x_d[mt * P:(mt + 1) * P, :])
                nc.vector.tensor_reduce(
                    eta_raw[:, mt:mt + 1], t[:], axis=AXIS.X, op=ALU.max,
                    apply_absolute_value=True,
                )
                nc.vector.tensor_scalar(
                    eta_all[:, mt:mt + 1], eta_raw[:, mt:mt + 1],
                    scalar1=1e-5, scalar2=None, op0=ALU.max,
                )
                nc.vector.reciprocal(inv_eta[:, mt:mt + 1], eta_all[:, mt:mt + 1])
                nc.vector.tensor_scalar(
                    qs_all[:, mt:mt + 1], inv_eta[:, mt:mt + 1],
                    scalar1=127.0, scalar2=None, op0=ALU.mult,
                )
                nc.scalar.activation(
                    t[:], t[:], ACTF.Copy, bias=C_MAGIC,
                    scale=qs_all[:, mt:mt + 1],
                )
                q = xqst.tile([P, K], BF16, tag="xq", name=f"xq{mt}")
                nc.vector.tensor_scalar(
                    q[:], t[:], scalar1=C_MAGIC, scalar2=None, op0=ALU.subtract,
                )
                xq_tiles[mt] = q
                if store_rt:
                    nc.scalar.dma_start(xq_rt_d[mt * P:(mt + 1) * P, :], q[:])
                # inv_osc row: transpose inv_eta column, scale by 127/gamma
                pt = ps_tr.tile([1, P], F32, tag="ptr", name=f"ios{mt}")
                nc.tensor.transpose(pt[:], inv_eta[:, mt:mt + 1], ident_f32[:])
                rs = slice(mt * P, (mt + 1) * P)
                nc.scalar.copy(ios_f32[:, rs], pt[:])
                nc.vector.tensor_scalar(
                    ios_f32[:, rs], ios_f32[:, rs],
                    scalar1=inv_g[0:1, :], scalar2=127.0,
                    op0=ALU.mult, op1=ALU.mult,
                )
                nc.vector.tensor_copy(ios_row[:, rs], ios_f32[:, rs])

            def x_transpose(mt):
                q = xq_tiles[mt]
                for g in range(4):
                    pt = ps_tr.tile([P, 512], BF16, tag="ptr", name=f"xt{mt}_{g}")
                    for j in range(4):
                        kt = g * 4 + j
                        nc.tensor.transpose(
                            pt[:, j * P:(j + 1) * P],
                            q[:, kt * P:(kt + 1) * P],
                            ident_sb[:],
                        )
                    dst = xqT_3d[:, g * 4:(g + 1) * 4, mt * P:(mt + 1) * P]
                    src = pt[:].rearrange("p (j m) -> p j m", j=4)
                    if g % 2 == 0:
                        nc.scalar.copy(dst, src)
                    else:
                        nc.vector.tensor_copy(dst, src)

            def xbar_load(r0, r1):
                for kt in range(KT):
                    nc.sync.dma_start_transpose(
                        xqT_3d[:, kt, r0:r1],
                        xq_rt_d[r0:r1, kt * P:(kt + 1) * P],
                    )

            def mm_block(mt):
                nc.vector.tensor_scalar(
                    osc_all[:, mt:mt + 1], eta_all[:, mt:mt + 1],
                    scalar1=gamma[:, :], scalar2=1.0 / 127.0,
                    op0=ALU.mult, op1=ALU.mult,
                )
                pss = [
                    ps_mm.tile([P, 512], F32, tag="psmm", name=f"ps{mt}_{nb}")
                    for nb in range(NBLK)
                ]
                for kt in range(KT):
                    lhsT = xqT_3d[:, kt, mt * P:(mt + 1) * P]
                    for nb in range(NBLK):
                        nc.tensor.matmul(
                            pss[nb][:],
                            lhsT,
                            wqT_3d[:, kt, nb * 512:(nb + 1) * 512],
                            start=(kt == 0),
                            stop=False,
                        )
                # bias as a rank-1 K=1 accumulation: psum += inv_osc[m]*bias[n]
                for nb in range(NBLK):
                    nc.tensor.matmul(
                        pss[nb][:],
                        ios_row[:, mt * P:(mt + 1) * P],
                        b_bf[:, nb * 512:(nb + 1) * 512],
                        start=False,
                        stop=True,
                    )
                for nb in range(NBLK):
                    o = outst.tile([P, 512], F32, tag="o", name=f"o{mt}_{nb}")
                    nc.scalar.activation(
                        o[:], pss[nb][:], ACTF.Copy, bias=0.0,
                        scale=osc_all[:, mt:mt + 1],
                    )
                    nc.sync.dma_start(
                        out_d[mt * P:(mt + 1) * P, nb * 512:(nb + 1) * 512], o[:]
                    )

            # software-pipelined: x chain runs one m-block ahead of matmuls
            for mt in range(MT):
                x_iter(mt, store_rt=(mt >= 4))
                if mt < 4:
                    x_transpose(mt)
                if mt == 7:
                    xbar_load(512, 1024)
                elif mt == 11:
                    xbar_load(1024, 1536)
                elif mt == 15:
                    xbar_load(1536, 2048)
                if mt >= 4:
                    mm_block(mt - 4)
            for mt in range(MT - 4, MT):
                mm_block(mt)
    nc.compile()
    return nc


_NC_CACHE = None
LAST_EXEC_NS = None


def _get_nc():
    global _NC_CACHE
    if _NC_CACHE is None:
        _NC_CACHE = _build_program()
    return _NC_CACHE


def _make_in_maps(x, weight, bias):
    xf = np.ascontiguousarray(np.asarray(x, dtype=np.float32).reshape(-1, K))
    w = np.ascontiguousarray(np.asarray(weight, dtype=np.float32))
    b = np.ascontiguousarray(np.asarray(bias, dtype=np.float32).reshape(1, N))
    assert xf.shape[0] == N_CORES * M_CORE
    return [
        {
            "x": xf[c * M_CORE:(c + 1) * M_CORE],
            "weight": w,
            "bias": b,
        }
        for c in range(N_CORES)
    ]


def kernel(x, weight, bias):
    global LAST_EXEC_NS
    nc = _get_nc()
    in_maps = _make_in_maps(x, weight, bias)
    trace = bool(int(os.environ.get("BITLINEAR_TRACE", "0")))
    res = run_bass_kernel_spmd(nc, in_maps, list(range(N_CORES)), trace=trace)
    LAST_EXEC_NS = res.exec_time_ns
    out = np.concatenate([res.results[c]["out"] for c in range(N_CORES)], axis=0)
    return out.reshape(np.asarray(x).shape[:-1] + (N,)).astype(np.float32)



# revision 18
# speedup vs baseline: 1.0652x; 1.0652x over previous
"""BitLinear (ternary-weight / int8-activation quantized linear) on 8 trn2 NeuronCores.

Math (matches the jax reference up to fp32 rounding):
    eta   = clip(max|x| along k, 1e-5)             per row
    x_q   = round(x * 127 / eta)    in [-127,127]  (round-half-even)
    gamma = clip(mean|w|, 1e-5)                    scalar
    w_q   = round(clip(w / gamma, -1, 1))          in {-1,0,1}
    out   = (x_q @ w_q^T) * (eta/127 * gamma) + bias

x_q / w_q are small integers exactly representable in bf16 and the PE
accumulates in fp32, so the bf16 matmul is EXACT.  Rounding uses the fp32
magic-number trick  rint(t) = (t + 1.5*2^23) - 1.5*2^23  (round-half-even).

Sharding: data-parallel over rows of x (16384 -> 2048 rows/core), weight+bias
replicated.

v2 design (vs v1 baseline at ~500us):
  - PE does ONLY the 1024 bf16 matmuls (no PE transposes, no bias matmuls).
  - All k-major transposes done by DMA xbar SBUF->SBUF (one instruction per
    [128,2048] bf16 tile, 3D strided dst), overlapped with matmuls.
  - w streamed with |w| reduce; first 4 n-tiles HELD in SBUF so they can be
    quantized the moment gamma is known; the other 12 re-read (pass2) on the
    gpsimd queue while matmuls on the held group already run.
  - bias added in fp32 via a partition-broadcast [128,N] tile (DVE
    scalar_tensor_tensor fused dequant+bias), not a rank-1 bf16 matmul.
  - elementwise work balanced across DVE / ACT / GpSimd.
"""

import os
from contextlib import ExitStack

import numpy as np

import concourse.bass as bass
import concourse.bacc as bacc
import concourse.mybir as mybir
import concourse.tile as tile
import concourse.bass_isa as bass_isa
from concourse.bass_utils import run_bass_kernel_spmd

P = 128
K = 2048
N = 2048
M_CORE = 2048
KT = K // P          # 16
NT = N // P          # 16
MT = M_CORE // P     # 16
NBLK = N // 512      # 4
N_CORES = 8
C_MAGIC = 12582912.0     # 1.5 * 2**23
INV_NK = 1.0 / (N * K)
N_HELD = 4               # w n-tiles held in SBUF across the gamma barrier

F32 = mybir.dt.float32
BF16 = mybir.dt.bfloat16
ALU = mybir.AluOpType
AXIS = mybir.AxisListType
ACTF = mybir.ActivationFunctionType


def _build_program():
    nc = bacc.Bacc("TRN2", target_bir_lowering=False, debug=False)

    x_d = nc.dram_tensor("x", [M_CORE, K], F32, kind="ExternalInput").ap()
    w_d = nc.dram_tensor("weight", [N, K], F32, kind="ExternalInput").ap()
    b_d = nc.dram_tensor("bias", [1, N], F32, kind="ExternalInput").ap()
    out_d = nc.dram_tensor("out", [M_CORE, N], F32, kind="ExternalOutput").ap()

    with tile.TileContext(nc) as tc, ExitStack() as ctx:
        stats = ctx.enter_context(tc.tile_pool(name="stats", bufs=1))
        bias_p = ctx.enter_context(tc.tile_pool(name="biasp", bufs=1))
        wqT_p = ctx.enter_context(tc.tile_pool(name="wqT", bufs=1))
        xqT_p = ctx.enter_context(tc.tile_pool(name="xqT", bufs=1))
        wstage = ctx.enter_context(tc.tile_pool(name="wstage", bufs=3))
        wqst = ctx.enter_context(tc.tile_pool(name="wqst", bufs=2))
        xstage = ctx.enter_context(tc.tile_pool(name="xstage", bufs=3))
        xqst = ctx.enter_context(tc.tile_pool(name="xqst", bufs=2))
        outst = ctx.enter_context(tc.tile_pool(name="outst", bufs=3))
        ps_mm = ctx.enter_context(
            tc.tile_pool(name="psmm", bufs=8, space=bass.MemorySpace.PSUM)
        )

        # ---- stats ----
        eta_c = stats.tile([P, MT], F32)
        inv_eta = stats.tile([P, MT], F32)
        qs_all = stats.tile([P, MT], F32)
        osc_all = stats.tile([P, MT], F32)
        wparts = stats.tile([P, NT + 3], F32)
        wsum = stats.tile([P, 1], F32)
        gsum = stats.tile([P, 1], F32)
        gamma = stats.tile([P, 1], F32)
        inv_g = stats.tile([P, 1], F32)

        # k-major quantized operands, [p, nt, kt, n-in-tile] so each w-tile
        # transpose writes a CONTIGUOUS block (strided xbar dst is broken on
        # HW); matmul rhs reads a 2-level strided [p, 4nt, 128] slice.
        wqT_all = wqT_p.tile([P, NT * KT * P], BF16)
        wqT_4d = wqT_all[:].rearrange("p (a t n) -> p a t n", a=NT, t=KT)
        xqT_all = xqT_p.tile([P, MT * KT * P], BF16)
        xqT_4d = xqT_all[:].rearrange("p (a t m) -> p a t m", a=MT, t=KT)

        bias_bc = bias_p.tile([P, N], F32)

        # ============ bias broadcast (uses an xstage buf as bounce) ========
        brow = xstage.tile([P, K], F32, tag="x", name="brow")
        nc.sync.dma_start(brow[0:1, 0:N], b_d[:, :])
        nc.gpsimd.partition_broadcast(bias_bc[:], brow[0:1, 0:N], channels=P)

        # ============ early x prefetch (tiles 0..2) ========================
        # x0..x2 stream + quantize while w pass1 owns most of the bandwidth,
        # so xqT[0..2] is ready long before the first matmul can start.
        x_tiles = {}
        xqT_tiles = {}

        def x_iter(mt):
            t = xstage.tile([P, K], F32, tag="x", name=f"x{mt}")
            nc.scalar.dma_start(t[:], x_d[mt * P:(mt + 1) * P, :])
            x_tiles[mt] = t

        def x_chain(mt, with_osc=True):
            t = x_tiles[mt]
            nc.vector.tensor_reduce(
                eta_c[:, mt:mt + 1], t[:], axis=AXIS.X, op=ALU.max,
                apply_absolute_value=True)
            nc.vector.tensor_scalar(
                eta_c[:, mt:mt + 1], eta_c[:, mt:mt + 1],
                scalar1=1e-5, scalar2=None, op0=ALU.max)
            nc.vector.reciprocal(inv_eta[:, mt:mt + 1], eta_c[:, mt:mt + 1])
            nc.vector.tensor_scalar(
                qs_all[:, mt:mt + 1], inv_eta[:, mt:mt + 1],
                scalar1=127.0, scalar2=None, op0=ALU.mult)
            if with_osc:
                osc_op(mt)
            nc.scalar.activation(
                t[:], t[:], ACTF.Copy, bias=C_MAGIC,
                scale=qs_all[:, mt:mt + 1])
            q = xqst.tile([P, K], BF16, tag="xq", name=f"xq{mt}")
            nc.vector.tensor_scalar(
                q[:], t[:], scalar1=C_MAGIC, scalar2=None, op0=ALU.subtract)
            # all xbar transposes MUST share one queue: two concurrent
            # DMA-transposes (even on different queues) corrupt each other
            nc.sync.dma_start_transpose(xqT_4d[:, mt, :, :], q[:])

        def osc_op(mt):
            nc.vector.tensor_scalar(
                osc_all[:, mt:mt + 1], eta_c[:, mt:mt + 1],
                scalar1=gamma[:, :], scalar2=1.0 / 127.0,
                op0=ALU.mult, op1=ALU.mult)

        N_EARLY = 3
        for mt in range(N_EARLY):
            x_iter(mt)

        # ============ w pass 1: stream, |w| partials; hold tiles 0..3 ======
        # order: 4..15 (rolling, discarded), then 0..3 (held; 3 reduced in
        # 4 chunks split across DVE/ACT so gamma lands right after arrival)
        for i, nt in enumerate(list(range(N_HELD, NT)) + list(range(N_HELD))):
            t = wstage.tile([P, K], F32, tag="w", name=f"w{nt}")
            eng = nc.sync if i % 2 == 0 else nc.scalar
            eng.dma_start(t[:], w_d[nt * P:(nt + 1) * P, :])
            if nt == N_HELD - 1:  # last-arriving tile: split the reduce
                # chunks -> cols [nt, NT, NT+1, NT+2]
                nc.vector.tensor_reduce(
                    wparts[:, nt:nt + 1], t[:, 0:512], axis=AXIS.X, op=ALU.add,
                    apply_absolute_value=True)
                nc.vector.tensor_reduce(
                    wparts[:, NT:NT + 1], t[:, 512:1024], axis=AXIS.X,
                    op=ALU.add, apply_absolute_value=True)
                for j in range(2):
                    scr = outst.tile([P, 512], F32, tag="o", name=f"scr{j}")
                    nc.scalar.activation(
                        scr[:], t[:, 1024 + j * 512:1024 + (j + 1) * 512],
                        ACTF.Abs, accum_out=wparts[:, NT + 1 + j:NT + 2 + j])
            else:
                # rolling tiles 4..15 -> cols 4..15; held 0..2 -> cols 0..2
                nc.vector.tensor_reduce(
                    wparts[:, nt:nt + 1], t[:], axis=AXIS.X, op=ALU.add,
                    apply_absolute_value=True)

        # ============ prefetch pass-2 loads (gpsimd) before gamma ==========
        pass2_tiles = {}

        def pass2_load(nt):
            t = wstage.tile([P, K], F32, tag="w", name=f"w2_{nt}")
            eng = nc.sync if nt % 2 == 0 else nc.scalar
            eng.dma_start(t[:], w_d[nt * P:(nt + 1) * P, :])
            pass2_tiles[nt] = t

        for nt in range(3):
            pass2_load(nt)

        # early x chains (no osc yet: gamma not computed)
        for mt in range(N_EARLY):
            x_chain(mt, with_osc=False)

        # ============ gamma ===============================================
        nc.vector.tensor_reduce(wsum[:], wparts[:], axis=AXIS.X, op=ALU.add)
        nc.gpsimd.partition_all_reduce(
            gsum[:], wsum[:], channels=P, reduce_op=bass_isa.ReduceOp.add)
        nc.vector.tensor_scalar(
            gamma[:], gsum[:], scalar1=INV_NK, scalar2=1e-5,
            op0=ALU.mult, op1=ALU.max)
        nc.vector.reciprocal(inv_g[:], gamma[:])

        # ============ w quantize + transpose ==============================
        def w_quant(nt, t):
            # t = w/gamma + C  (fp32 add rounds to integer grid, RNE)
            nc.scalar.activation(
                t[:], t[:], ACTF.Copy, bias=C_MAGIC, scale=inv_g[:, :])
            nc.vector.tensor_scalar(
                t[:], t[:], scalar1=C_MAGIC, scalar2=1.0,
                op0=ALU.subtract, op1=ALU.min)
            q = wqst.tile([P, K], BF16, tag="wq", name=f"wq{nt}")
            nc.vector.tensor_scalar(
                q[:], t[:], scalar1=-1.0, scalar2=None, op0=ALU.max)
            nc.sync.dma_start_transpose(wqT_4d[:, nt, :, :], q[:])

        # osc for the early x tiles (needs gamma)
        for mt in range(N_EARLY):
            osc_op(mt)

        # ============ pass-2 w quantize ====================================
        for nt in range(NT):
            w_quant(nt, pass2_tiles[nt])
            if nt + 3 < NT:
                pass2_load(nt + 3)

        # ============ x pipeline + matmuls =================================
        def mm_block(mt):
            for nb in range(NBLK):
                ps = ps_mm.tile([P, 512], F32, tag="ps", name=f"ps{mt}_{nb}")
                for kt in range(KT):
                    nc.tensor.matmul(
                        ps[:],
                        xqT_4d[:, mt, kt, :],
                        wqT_4d[:, nb * 4:(nb + 1) * 4, kt, :],
                        start=(kt == 0),
                        stop=(kt == KT - 1),
                    )
                o = outst.tile([P, 512], F32, tag="o", name=f"o{mt}_{nb}")
                osc = osc_all[:, mt:mt + 1]
                bsl = bias_bc[:, nb * 512:(nb + 1) * 512]
                nc.vector.scalar_tensor_tensor(
                    o[:], ps[:], osc, bsl, op0=ALU.mult, op1=ALU.add)
                eng = nc.scalar if (mt + nb) % 2 == 0 else nc.gpsimd
                eng.dma_start(
                    out_d[mt * P:(mt + 1) * P, nb * 512:(nb + 1) * 512], o[:])

        for s in range(N_EARLY, MT):
            x_iter(s)
            if s >= N_EARLY + 2:
                mm_block(s - N_EARLY - 2)
            x_chain(s)
        for mt in range(MT - N_EARLY - 2, MT):
            mm_block(mt)

    nc.compile()
    return nc


_NC_CACHE = None
LAST_EXEC_NS = None


def _get_nc():
    global _NC_CACHE
    if _NC_CACHE is None:
        _NC_CACHE = _build_program()
    return _NC_CACHE


def _make_in_maps(x, weight, bias):
    xf = np.ascontiguousarray(np.asarray(x, dtype=np.float32).reshape(-1, K))
    w = np.ascontiguousarray(np.asarray(weight, dtype=np.float32))
    b = np.ascontiguousarray(np.asarray(bias, dtype=np.float32).reshape(1, N))
    assert xf.shape[0] == N_CORES * M_CORE
    return [
        {
            "x": xf[c * M_CORE:(c + 1) * M_CORE],
            "weight": w,
            "bias": b,
        }
        for c in range(N_CORES)
    ]


def kernel(x, weight, bias):
    global LAST_EXEC_NS
    nc = _get_nc()
    in_maps = _make_in_maps(x, weight, bias)
    trace = bool(int(os.environ.get("BITLINEAR_TRACE", "0")))
    res = run_bass_kernel_spmd(nc, in_maps, list(range(N_CORES)), trace=trace)
    LAST_EXEC_NS = res.exec_time_ns
    out = np.concatenate([res.results[c]["out"] for c in range(N_CORES)], axis=0)
    return out.reshape(np.asarray(x).shape[:-1] + (N,)).astype(np.float32)


# revision 19
# speedup vs baseline: 1.0680x; 1.0026x over previous
"""BitLinear (ternary-weight / int8-activation quantized linear) on 8 trn2 NeuronCores.

Math (matches the jax reference up to fp32 rounding):
    eta   = clip(max|x| along k, 1e-5)             per row
    x_q   = round(x * 127 / eta)    in [-127,127]  (round-half-even)
    gamma = clip(mean|w|, 1e-5)                    scalar
    w_q   = round(clip(w / gamma, -1, 1))          in {-1,0,1}
    out   = (x_q @ w_q^T) * (eta/127 * gamma) + bias

x_q / w_q are small integers exactly representable in bf16 and the PE
accumulates in fp32, so the bf16 matmul is EXACT.  Rounding uses the fp32
magic-number trick  rint(t) = (t + 1.5*2^23) - 1.5*2^23  (round-half-even).

Sharding: data-parallel over rows of x (16384 -> 2048 rows/core), weight+bias
replicated.

v2 design (vs v1 baseline at ~500us):
  - PE does ONLY the 1024 bf16 matmuls (no PE transposes, no bias matmuls).
  - All k-major transposes done by DMA xbar SBUF->SBUF (one instruction per
    [128,2048] bf16 tile, 3D strided dst), overlapped with matmuls.
  - w streamed with |w| reduce; first 4 n-tiles HELD in SBUF so they can be
    quantized the moment gamma is known; the other 12 re-read (pass2) on the
    gpsimd queue while matmuls on the held group already run.
  - bias added in fp32 via a partition-broadcast [128,N] tile (DVE
    scalar_tensor_tensor fused dequant+bias), not a rank-1 bf16 matmul.
  - elementwise work balanced across DVE / ACT / GpSimd.
"""

import os
from contextlib import ExitStack

import numpy as np

import concourse.bass as bass
import concourse.bacc as bacc
import concourse.mybir as mybir
import concourse.tile as tile
import concourse.bass_isa as bass_isa
from concourse.bass_utils import run_bass_kernel_spmd

P = 128
K = 2048
N = 2048
M_CORE = 2048
KT = K // P          # 16
NT = N // P          # 16
MT = M_CORE // P     # 16
NBLK = N // 512      # 4
N_CORES = 8
C_MAGIC = 12582912.0     # 1.5 * 2**23
INV_NK = 1.0 / (N * K)
N_HELD = 4               # w n-tiles held in SBUF across the gamma barrier

F32 = mybir.dt.float32
BF16 = mybir.dt.bfloat16
ALU = mybir.AluOpType
AXIS = mybir.AxisListType
ACTF = mybir.ActivationFunctionType


def _build_program():
    nc = bacc.Bacc("TRN2", target_bir_lowering=False, debug=False)

    x_d = nc.dram_tensor("x", [M_CORE, K], F32, kind="ExternalInput").ap()
    w_d = nc.dram_tensor("weight", [N, K], F32, kind="ExternalInput").ap()
    b_d = nc.dram_tensor("bias", [1, N], F32, kind="ExternalInput").ap()
    out_d = nc.dram_tensor("out", [M_CORE, N], F32, kind="ExternalOutput").ap()

    with tile.TileContext(nc) as tc, ExitStack() as ctx:
        stats = ctx.enter_context(tc.tile_pool(name="stats", bufs=1))
        bias_p = ctx.enter_context(tc.tile_pool(name="biasp", bufs=1))
        wqT_p = ctx.enter_context(tc.tile_pool(name="wqT", bufs=1))
        xqT_p = ctx.enter_context(tc.tile_pool(name="xqT", bufs=1))
        wstage = ctx.enter_context(tc.tile_pool(name="wstage", bufs=3))
        wqst = ctx.enter_context(tc.tile_pool(name="wqst", bufs=2))
        xstage = ctx.enter_context(tc.tile_pool(name="xstage", bufs=3))
        xqst = ctx.enter_context(tc.tile_pool(name="xqst", bufs=2))
        outst = ctx.enter_context(tc.tile_pool(name="outst", bufs=3))
        ps_mm = ctx.enter_context(
            tc.tile_pool(name="psmm", bufs=8, space=bass.MemorySpace.PSUM)
        )

        # ---- stats ----
        eta_c = stats.tile([P, MT], F32)
        inv_eta = stats.tile([P, MT], F32)
        qs_all = stats.tile([P, MT], F32)
        osc_all = stats.tile([P, MT], F32)
        wparts = stats.tile([P, NT + 3], F32)
        wsum = stats.tile([P, 1], F32)
        gsum = stats.tile([P, 1], F32)
        gamma = stats.tile([P, 1], F32)
        inv_g = stats.tile([P, 1], F32)

        # k-major quantized operands, [p, nt, kt, n-in-tile] so each w-tile
        # transpose writes a CONTIGUOUS block (strided xbar dst is broken on
        # HW); matmul rhs reads a 2-level strided [p, 4nt, 128] slice.
        wqT_all = wqT_p.tile([P, NT * KT * P], BF16)
        wqT_4d = wqT_all[:].rearrange("p (a t n) -> p a t n", a=NT, t=KT)
        xqT_all = xqT_p.tile([P, MT * KT * P], BF16)
        xqT_4d = xqT_all[:].rearrange("p (a t m) -> p a t m", a=MT, t=KT)

        bias_bc = bias_p.tile([P, N], F32)

        # ============ bias broadcast (uses an xstage buf as bounce) ========
        brow = xstage.tile([P, K], F32, tag="x", name="brow")
        nc.sync.dma_start(brow[0:1, 0:N], b_d[:, :])
        nc.gpsimd.partition_broadcast(bias_bc[:], brow[0:1, 0:N], channels=P)

        # ============ early x prefetch (tiles 0..2) ========================
        # x0..x2 stream + quantize while w pass1 owns most of the bandwidth,
        # so xqT[0..2] is ready long before the first matmul can start.
        x_tiles = {}
        xqT_tiles = {}

        def x_iter(mt):
            t = xstage.tile([P, K], F32, tag="x", name=f"x{mt}")
            nc.scalar.dma_start(t[:], x_d[mt * P:(mt + 1) * P, :])
            x_tiles[mt] = t

        def x_chain(mt, with_osc=True):
            t = x_tiles[mt]
            nc.vector.tensor_reduce(
                eta_c[:, mt:mt + 1], t[:], axis=AXIS.X, op=ALU.max,
                apply_absolute_value=True)
            nc.vector.tensor_scalar(
                eta_c[:, mt:mt + 1], eta_c[:, mt:mt + 1],
                scalar1=1e-5, scalar2=None, op0=ALU.max)
            nc.vector.reciprocal(inv_eta[:, mt:mt + 1], eta_c[:, mt:mt + 1])
            nc.vector.tensor_scalar(
                qs_all[:, mt:mt + 1], inv_eta[:, mt:mt + 1],
                scalar1=127.0, scalar2=None, op0=ALU.mult)
            if with_osc:
                osc_op(mt)
            nc.scalar.activation(
                t[:], t[:], ACTF.Copy, bias=C_MAGIC,
                scale=qs_all[:, mt:mt + 1])
            q = xqst.tile([P, K], BF16, tag="xq", name=f"xq{mt}")
            nc.vector.tensor_scalar(
                q[:], t[:], scalar1=C_MAGIC, scalar2=None, op0=ALU.subtract)
            # all xbar transposes MUST share one queue: two concurrent
            # DMA-transposes (even on different queues) corrupt each other
            nc.sync.dma_start_transpose(xqT_4d[:, mt, :, :], q[:])

        def osc_op(mt):
            nc.vector.tensor_scalar(
                osc_all[:, mt:mt + 1], eta_c[:, mt:mt + 1],
                scalar1=gamma[:, :], scalar2=1.0 / 127.0,
                op0=ALU.mult, op1=ALU.mult)

        N_EARLY = 3
        for mt in range(N_EARLY):
            x_iter(mt)
        # early x chains emitted FIRST so DVE/ACT process them immediately
        # (no osc yet: gamma not computed)
        for mt in range(N_EARLY):
            x_chain(mt, with_osc=False)

        # ============ w pass 1: stream, |w| partials (ACT Abs in-place) ====
        # order: 4..15 first, then 0..3; the last tile's reduce is split in
        # 4 chunks across DVE/ACT so gamma lands right after its arrival
        for i, nt in enumerate(list(range(N_HELD, NT)) + list(range(N_HELD))):
            t = wstage.tile([P, K], F32, tag="w", name=f"w{nt}")
            eng = nc.scalar if i % 2 == 0 else nc.gpsimd
            eng.dma_start(t[:], w_d[nt * P:(nt + 1) * P, :])
            if nt == N_HELD - 1:  # last-arriving tile: split the reduce
                # chunks -> cols [nt, NT, NT+1, NT+2]
                nc.vector.tensor_reduce(
                    wparts[:, nt:nt + 1], t[:, 0:512], axis=AXIS.X, op=ALU.add,
                    apply_absolute_value=True)
                nc.vector.tensor_reduce(
                    wparts[:, NT:NT + 1], t[:, 512:1024], axis=AXIS.X,
                    op=ALU.add, apply_absolute_value=True)
                for j in range(2):
                    nc.scalar.activation(
                        t[:, 1024 + j * 512:1024 + (j + 1) * 512],
                        t[:, 1024 + j * 512:1024 + (j + 1) * 512],
                        ACTF.Abs, accum_out=wparts[:, NT + 1 + j:NT + 2 + j])
            else:
                # |w| partials on ACT (in-place, tile is discarded) keeps DVE
                # free for the x pipeline
                nc.scalar.activation(
                    t[:], t[:], ACTF.Abs,
                    accum_out=wparts[:, nt:nt + 1])

        # ============ prefetch pass-2 loads (gpsimd ring) before gamma =====
        pass2_tiles = {}

        def pass2_load(nt):
            t = wstage.tile([P, K], F32, tag="w", name=f"w2_{nt}")
            nc.gpsimd.dma_start(t[:], w_d[nt * P:(nt + 1) * P, :])
            pass2_tiles[nt] = t

        for nt in range(6):
            pass2_load(nt)

        # ============ gamma ===============================================
        nc.vector.tensor_reduce(wsum[:], wparts[:], axis=AXIS.X, op=ALU.add)
        nc.gpsimd.partition_all_reduce(
            gsum[:], wsum[:], channels=P, reduce_op=bass_isa.ReduceOp.add)
        nc.vector.tensor_scalar(
            gamma[:], gsum[:], scalar1=INV_NK, scalar2=1e-5,
            op0=ALU.mult, op1=ALU.max)
        nc.vector.reciprocal(inv_g[:], gamma[:])

        # ============ w quantize + transpose ==============================
        def w_quant(nt, t):
            # t = w/gamma + C  (fp32 add rounds to integer grid, RNE)
            nc.scalar.activation(
                t[:], t[:], ACTF.Copy, bias=C_MAGIC, scale=inv_g[:, :])
            nc.vector.tensor_scalar(
                t[:], t[:], scalar1=C_MAGIC, scalar2=1.0,
                op0=ALU.subtract, op1=ALU.min)
            q = wqst.tile([P, K], BF16, tag="wq", name=f"wq{nt}")
            nc.vector.tensor_scalar(
                q[:], t[:], scalar1=-1.0, scalar2=None, op0=ALU.max)
            nc.sync.dma_start_transpose(wqT_4d[:, nt, :, :], q[:])

        # osc for the early x tiles (needs gamma)
        for mt in range(N_EARLY):
            osc_op(mt)

        # ============ pass-2 w quantize ====================================
        for nt in range(NT):
            w_quant(nt, pass2_tiles[nt])
            if nt + 6 < NT:
                pass2_load(nt + 6)

        # ============ x pipeline + matmuls =================================
        def mm_group(mt, nb):
            ps = ps_mm.tile([P, 512], F32, tag="ps", name=f"ps{mt}_{nb}")
            for kt in range(KT):
                nc.tensor.matmul(
                    ps[:],
                    xqT_4d[:, mt, kt, :],
                    wqT_4d[:, nb * 4:(nb + 1) * 4, kt, :],
                    start=(kt == 0),
                    stop=(kt == KT - 1),
                )
            o = outst.tile([P, 512], F32, tag="o", name=f"o{mt}_{nb}")
            osc = osc_all[:, mt:mt + 1]
            bsl = bias_bc[:, nb * 512:(nb + 1) * 512]
            nc.vector.scalar_tensor_tensor(
                o[:], ps[:], osc, bsl, op0=ALU.mult, op1=ALU.add)
            eng = nc.scalar if (mt + nb) % 2 == 0 else nc.gpsimd
            eng.dma_start(
                out_d[mt * P:(mt + 1) * P, nb * 512:(nb + 1) * 512], o[:])

        # wavefront over the first 3 m-tiles: nb groups arrive over time as
        # pass2 tiles are quantized, so visit (mt, nb) in nb-major order to
        # avoid PE-queue head-of-line stalls on not-yet-ready nb groups
        for nb in range(NBLK):
            for mt in range(N_EARLY):
                mm_group(mt, nb)

        for s in range(N_EARLY, MT):
            x_iter(s)
            if s >= N_EARLY + 2:
                for nb in range(NBLK):
                    mm_group(s - 2, nb)
            x_chain(s)
        for mt in range(MT - 2, MT):
            for nb in range(NBLK):
                mm_group(mt, nb)

    nc.compile()
    return nc


_NC_CACHE = None
LAST_EXEC_NS = None


def _get_nc():
    global _NC_CACHE
    if _NC_CACHE is None:
        _NC_CACHE = _build_program()
    return _NC_CACHE


def _make_in_maps(x, weight, bias):
    xf = np.ascontiguousarray(np.asarray(x, dtype=np.float32).reshape(-1, K))
    w = np.ascontiguousarray(np.asarray(weight, dtype=np.float32))
    b = np.ascontiguousarray(np.asarray(bias, dtype=np.float32).reshape(1, N))
    assert xf.shape[0] == N_CORES * M_CORE
    return [
        {
            "x": xf[c * M_CORE:(c + 1) * M_CORE],
            "weight": w,
            "bias": b,
        }
        for c in range(N_CORES)
    ]


def kernel(x, weight, bias):
    global LAST_EXEC_NS
    nc = _get_nc()
    in_maps = _make_in_maps(x, weight, bias)
    trace = bool(int(os.environ.get("BITLINEAR_TRACE", "0")))
    res = run_bass_kernel_spmd(nc, in_maps, list(range(N_CORES)), trace=trace)
    LAST_EXEC_NS = res.exec_time_ns
    out = np.concatenate([res.results[c]["out"] for c in range(N_CORES)], axis=0)
    return out.reshape(np.asarray(x).shape[:-1] + (N,)).astype(np.float32)
